# revision 9
# baseline (speedup 1.0000x reference)
"""Trainium2 Bass kernel for nn_MixtureAttention.

Math: the reference builds a (c,c) pairwise Cauchy-product matrix per batch,
row-normalizes it, and keeps only the diagonal.  `pi` cancels; the kept
diagonal reduces to
    coef[i] = (1/DENOM[i,i]) / sum_p 1/DENOM[i,p]
with DENOM[i,p] = prod_d (sig[i,d]^2 + (mu[p,d]-mu[i,d])^2), and
y[b,ch,i] = x[b,ch] * coef[b,i].

Kernel strategy: DENOM factors into two dim-pair products, each a degree-2x2
polynomial in the (centered) point coords -> a rank-9 matmul
    pair[i,p] = sum_t F[i,t] * G[t,p]
run on the PE at fp32r speed (1 col/cycle).  fp32r truncates inputs to FP22
but multiplies exactly and accumulates fp32, so a hi/lo split
(F = F + F2, G = G + G2 with F2 = F - trunc22(F)) stacked into K=36 recovers
fp32-level accuracy at fp32r speed.  Per (128-row, 512-col) tile:
  - PE:  2 fp32r matmuls (K=36) -> PSUM pair products
  - DVE: one tensor_tensor mult -> DENOM strip in SBUF
  - ACT: one Reciprocal pass over the 4096-wide strip whose accum_out
    carries the row-sum
The matmul-computed diagonal entry (DENOM[i,i] ~ prod sig^4, worst-case
cancellation) is patched exactly: a masked affine_mul_reduce extracts the
computed 1/DENOM[i,i], the epilogue swaps in the exact 1/prod(sig^2).
Host pre-rotates the point columns per core so the diagonal block sits at a
static offset (the row-sum is order-invariant).

Sharding: 8 cores; core k handles batch k//2, c-rows [ (k%2)*2048, +2048 ).
"""

import numpy as np

B, C, D, CH = 4, 4096, 4, 256
NCORES = 8
CW = C // 2            # 2048 c-rows per core (2 cores per batch)
NBLK = CW // 128       # 16 row blocks
PCH = 512              # matmul free-dim chunk (one PSUM bank)
NPCH = C // PCH        # 8 chunks per row-block strip
K = 36                 # stacked contraction: [F;F2;F;F2] x [G;G;G2;G2]
NOUT = 512             # matmul free-dim tile for the output outer product

_cache = {}


def _build(bench_nrep=None, bench_span="main"):
    import concourse.bacc as bacc
    import concourse.mybir as mybir
    from concourse.tile import TileContext

    f32 = mybir.dt.float32
    f32r = mybir.dt.float32r
    Alu = mybir.AluOpType
    Act = mybir.ActivationFunctionType

    nc = bacc.Bacc(None, target_bir_lowering=False)
    g0_r = nc.declare_dram_parameter("g0_r", [K, C], f32, isOutput=False)
    g1_r = nc.declare_dram_parameter("g1_r", [K, C], f32, isOutput=False)
    f0_r = nc.declare_dram_parameter("f0_r", [K, CW], f32, isOutput=False)
    f1_r = nc.declare_dram_parameter("f1_r", [K, CW], f32, isOutput=False)
    ps2i_r = nc.declare_dram_parameter("ps2i_r", [128, NBLK], f32, isOutput=False)
    ident_r = nc.declare_dram_parameter("ident_r", [128, 128], f32, isOutput=False)
    xv = nc.declare_dram_parameter("xv", [1, CH], f32, isOutput=False)
    y = nc.declare_dram_parameter("y", [CH, CW], f32, isOutput=True)

    with TileContext(nc) as tc:
        with (
            tc.tile_pool(name="persist", bufs=1) as pp,
            tc.tile_pool(name="work", bufs=1) as wp,
            tc.tile_pool(name="psum", bufs=1, space="PSUM") as psp,
            tc.tile_pool(name="dram", bufs=1, space="DRAM") as dp,
        ):
            scr = dp.tile([128 * NBLK], f32, name="scr")
            ps2i_sb = pp.tile([128, NBLK], f32)
            nc.sync.dma_start(out=ps2i_sb[:, :], in_=ps2i_r[:, :])
            ident = pp.tile([128, 128], f32)
            nc.sync.dma_start(out=ident[:, :], in_=ident_r[:, :])
            xv_sb = pp.tile([1, CH], f32)
            nc.sync.dma_start(out=xv_sb[0:1, :], in_=xv[0:1, :])

            g0 = pp.tile([K, C], f32r)
            g1 = pp.tile([K, C], f32r)
            f0 = pp.tile([K, CW], f32r)
            f1 = pp.tile([K, CW], f32r)

            Racc = pp.tile([128, NBLK], f32)
            Rdiag = pp.tile([128, NBLK], f32)

            def load_loop():
                # F first halves, then G chunks in consumption order,
                # then F second halves.
                nc.sync.dma_start(out=f0[:, 0:CW // 2], in_=f0_r[:, 0:CW // 2].bitcast(f32r))
                nc.sync.dma_start(out=f1[:, 0:CW // 2], in_=f1_r[:, 0:CW // 2].bitcast(f32r))
                for j in range(NPCH):
                    sl = slice(j * PCH, (j + 1) * PCH)
                    nc.sync.dma_start(out=g0[:, sl], in_=g0_r[:, sl].bitcast(f32r))
                    nc.sync.dma_start(out=g1[:, sl], in_=g1_r[:, sl].bitcast(f32r))
                nc.sync.dma_start(out=f0[:, CW // 2:], in_=f0_r[:, CW // 2:].bitcast(f32r))
                nc.sync.dma_start(out=f1[:, CW // 2:], in_=f1_r[:, CW // 2:].bitcast(f32r))

            DVE_COPY_J = (2, 5)   # which chunks' PSUM->SBUF stage runs on DVE

            def main_loop(n_lo, n_hi):
              for n in range(n_lo, n_hi):
                den = wp.tile([128, C], f32, tag="den", bufs=2, name="den")
                rr = wp.tile([128, C], f32, tag="rr", bufs=2, name="rr")
                fa = f0[:, n * 128:(n + 1) * 128]
                fb = f1[:, n * 128:(n + 1) * 128]
                for j in range(NPCH):
                    sl = slice(j * PCH, (j + 1) * PCH)
                    pa = psp.tile([128, PCH], f32, tag="pa", bufs=3, name="pa")
                    nc.tensor.matmul(
                        pa[:, :], fa, g0[:, sl],
                        start=True, stop=True,
                    )
                    pb = psp.tile([128, PCH], f32, tag="pb", bufs=3, name="pb")
                    nc.tensor.matmul(
                        pb[:, :], fb, g1[:, sl],
                        start=True, stop=True,
                    )
                    # stage pa to SBUF (DVE can read at most one PSUM operand)
                    sa = wp.tile([128, PCH], f32, tag="sa", bufs=3, name="sa")
                    if j in DVE_COPY_J:
                        nc.vector.tensor_copy(sa[:, :], pa[:, :])
                    else:
                        nc.scalar.copy(sa[:, :], pa[:, :])
                    nc.vector.tensor_tensor(
                        den[:, sl], sa[:, :], pb[:, :], Alu.mult
                    )
                # reciprocal + row-sum over the whole 4096 strip on ACT
                imm = lambda v: mybir.ImmediateValue(dtype=mybir.dt.float32, value=v)
                eng = nc.scalar
                eng.add_instruction(
                    mybir.InstActivation(
                        name=nc.get_next_instruction_name(),
                        func=Act.Reciprocal,
                        ins=[
                            eng.lower_ap(den[:, :]),
                            imm(0.0), imm(1.0), imm(0.0),
                        ],
                        outs=[
                            eng.lower_ap(rr[:, :]),
                            eng.lower_ap(Racc[:, n:n + 1]),
                        ],
                    )
                )
                # extract the computed 1/DENOM[i,i] (diag block is at a
                # static offset thanks to the host-side column rotation)
                junk = wp.tile([128, 128], f32, tag="junk", bufs=2, name="junk")
                nc.vector.affine_mul_reduce(
                    out=junk[:, :], accum_out=Rdiag[:, n:n + 1],
                    in0=rr[:, n * 128:(n + 1) * 128], in1=ident[:, :],
                    scale=1.0, bias=0.0,
                )

            HB = NBLK // 2          # blocks per epilogue half
            HC = HB * 128           # c-columns per half

            def epilogue(half):
                nsl = slice(half * HB, (half + 1) * HB)
                Tc = pp.tile([128, HB], f32, name="Tc", tag="Tc", bufs=2)
                nc.vector.tensor_tensor(
                    Tc[:, :], Racc[:, nsl], Rdiag[:, nsl], Alu.subtract
                )
                nc.vector.tensor_tensor(
                    Tc[:, :], Tc[:, :], ps2i_sb[:, nsl], Alu.add
                )
                coef = pp.tile([128, HB], f32, name="coef", tag="coef", bufs=2)
                nc.vector.reciprocal(coef[:, :], Tc[:, :])
                nc.vector.tensor_tensor(
                    coef[:, :], coef[:, :], ps2i_sb[:, nsl], Alu.mult
                )

                # transpose coef (128, HB) -> row (1, HC) via a DRAM bounce
                nc.sync.dma_start(
                    out=scr.rearrange("(p n) -> p n", p=128)[:, nsl], in_=coef[:, :]
                )
                crow = pp.tile([1, HC], f32, name="crow", tag="crow", bufs=2)
                nc.sync.dma_start(
                    out=crow[0:1, :].rearrange("a (n p) -> a n p", n=HB),
                    in_=scr.rearrange("(p n) -> n p", n=NBLK)[nsl, :],
                )

                # y[ch, c] = x[ch] * coef[c] as K=1 outer-product matmuls
                for h in range(CH // 128):
                    for qk in range(HC // NOUT):
                        ps = psp.tile([128, NOUT], f32, tag="ps", bufs=2, name="ps")
                        nc.tensor.matmul(
                            ps[:, :],
                            xv_sb[0:1, h * 128:(h + 1) * 128],
                            crow[0:1, qk * NOUT:(qk + 1) * NOUT],
                            start=True, stop=True,
                        )
                        ysb = wp.tile([128, NOUT], f32, tag="ysb", bufs=2, name="ysb")
                        nc.scalar.copy(ysb[:, :], ps[:, :])
                        nc.sync.dma_start(
                            out=y[
                                h * 128:(h + 1) * 128,
                                half * HC + qk * NOUT: half * HC + (qk + 1) * NOUT,
                            ],
                            in_=ysb[:, :],
                        )

            def whole():
                load_loop()
                main_loop(0, NBLK // 2)
                epilogue(0)
                main_loop(NBLK // 2, NBLK)
                epilogue(1)

            if bench_nrep is None:
                whole()
            elif bench_span == "main":
                load_loop()
                with tc.For_i(0, bench_nrep, 1):
                    main_loop(0, NBLK)
                epilogue(0)
                epilogue(1)
            elif bench_span == "load":
                with tc.For_i(0, bench_nrep, 1):
                    load_loop()
                main_loop(0, NBLK)
                epilogue(0)
                epilogue(1)
            elif bench_span == "epi":
                load_loop()
                main_loop(0, NBLK)
                with tc.For_i(0, bench_nrep, 1):
                    epilogue(0)
                    epilogue(1)
            else:
                import concourse.mybir as _mb

                with tc.For_i(
                    0, bench_nrep, 1,
                    staggered_reset=True,
                    hint_engines=(_mb.EngineType.DVE, _mb.EngineType.Activation),
                ):
                    whole()
    nc.finalize()
    return nc


def _get_nc():
    if "nc" not in _cache:
        _cache["nc"] = _build()
    return _cache["nc"]


def _trunc22(a):
    a = np.ascontiguousarray(a, dtype=np.float32)
    return (a.view(np.uint32) & np.uint32(0xFFFFF000)).view(np.float32)


def _in_maps(x, mu, sig):
    maps = []
    ident = np.eye(128, dtype=np.float32)
    for k in range(NCORES):
        b = k // 2
        half = k % 2
        sl = slice(half * CW, (half + 1) * CW)
        muc = (np.asarray(mu[b], np.float32) - np.float32(0.5)).astype(np.float32)
        mur = muc[sl]                                    # (CW, 4) row centers
        sgr = np.asarray(sig[b, sl], dtype=np.float32)   # (CW, 4)

        gs, fs = [], []
        for (da, db) in ((0, 1), (2, 3)):
            xa, xb = muc[:, da], muc[:, db]
            one = np.ones_like(xa)
            ga = [one, xa, (xa * xa).astype(np.float32)]
            gb = [one, xb, (xb * xb).astype(np.float32)]
            G = np.stack(
                [(ga[ja] * gb[jb]).astype(np.float32)
                 for ja in range(3) for jb in range(3)], 0)       # (9, C)

            def cvec(dd):
                s2 = (sgr[:, dd] * sgr[:, dd]).astype(np.float32)
                m = mur[:, dd]
                return [(s2 + m * m).astype(np.float32),
                        (np.float32(-2.0) * m).astype(np.float32),
                        np.ones_like(m)]

            ca, cb = cvec(da), cvec(db)
            F = np.stack(
                [(ca[ja] * cb[jb]).astype(np.float32)
                 for ja in range(3) for jb in range(3)], 0)       # (9, CW)
            G1 = _trunc22(G)
            F1 = _trunc22(F)
            G2 = (G - G1).astype(np.float32)
            F2 = (F - F1).astype(np.float32)
            # K segments: [F1,G1] [F2,G1] [F1,G2] [F2,G2]; hi rows are
            # pre-truncated so the DMA's fp32->fp32r rounding is an identity
            gstack = np.concatenate([G1, G1, G2, G2], 0)          # (36, C)
            fstack = np.concatenate([F1, F2, F1, F2], 0)          # (36, CW)
            # rotate points so each core's diagonal block is at col 128*n
            gstack = np.roll(gstack, -half * CW, axis=1)
            gs.append(np.ascontiguousarray(gstack))
            fs.append(np.ascontiguousarray(fstack))

        ps2 = (sgr * sgr).astype(np.float32).prod(axis=1).astype(np.float32)
        ps2i = (np.float32(1.0) / ps2).astype(np.float32)         # exact 1/DENOM_ii
        ps2i_rr = np.ascontiguousarray(ps2i.reshape(NBLK, 128).T)

        maps.append(
            {
                "g0_r": gs[0], "g1_r": gs[1],
                "f0_r": fs[0], "f1_r": fs[1],
                "ps2i_r": ps2i_rr,
                "ident_r": ident,
                "xv": np.ascontiguousarray(
                    np.asarray(x[b, :, 0], dtype=np.float32)[None, :]
                ),
            }
        )
    return maps


def kernel(x, pi, mu, sig):
    from concourse.bass_utils import run_bass_kernel_spmd

    nc = _get_nc()
    res = run_bass_kernel_spmd(nc, _in_maps(x, mu, sig), list(range(NCORES))).results
    y = np.empty((B, CH, C), np.float32)
    for k in range(NCORES):
        b = k // 2
        half = k % 2
        y[b, :, half * CW:(half + 1) * CW] = res[k]["y"]
    return y


# revision 13
# speedup vs baseline: 1.2221x; 1.2221x over previous
"""Trainium2 Bass kernel for nn_MixtureAttention.

Math: the reference builds a (c,c) pairwise Cauchy-product matrix per batch,
row-normalizes it, and keeps only the diagonal.  `pi` cancels; the kept
diagonal reduces to
    coef[i] = (1/DENOM[i,i]) / sum_p 1/DENOM[i,p]
with DENOM[i,p] = prod_d (sig[i,d]^2 + (mu[p,d]-mu[i,d])^2), and
y[b,ch,i] = x[b,ch] * coef[b,i].

Kernel strategy: DENOM factors into two dim-pair products, each a degree-2x2
polynomial in the (centered) point coords -> a rank-9 matmul
    pair[i,p] = sum_t F[i,t] * G[t,p]
run on the PE at fp32r speed (1 col/cycle).  fp32r truncates inputs to FP22
but multiplies exactly and accumulates fp32, so a hi/lo split
(F = F + F2, G = G + G2 with F2 = F - trunc22(F)) stacked into K=36 recovers
fp32-level accuracy at fp32r speed.  Per (128-row, 512-col) tile:
  - PE:  2 fp32r matmuls (K=36) -> PSUM pair products
  - DVE: one tensor_tensor mult -> DENOM strip in SBUF
  - ACT: one Reciprocal pass over the 4096-wide strip whose accum_out
    carries the row-sum
The matmul-computed diagonal entry (DENOM[i,i] ~ prod sig^4, worst-case
cancellation) is patched exactly: a masked affine_mul_reduce extracts the
computed 1/DENOM[i,i], the epilogue swaps in the exact 1/prod(sig^2).
Host pre-rotates the point columns per core so the diagonal block sits at a
static offset (the row-sum is order-invariant).

Sharding: 8 cores; core k handles batch k//2, c-rows [ (k%2)*2048, +2048 ).
"""

import numpy as np

B, C, D, CH = 4, 4096, 4, 256
NCORES = 8
CW = C // 2            # 2048 c-rows per core (2 cores per batch)
NBLK = CW // 128       # 16 row blocks
PCH = 512              # matmul free-dim chunk (one PSUM bank)
NPCH = C // PCH        # 8 chunks per row-block strip
K = 36                 # stacked contraction: [F;F2;F;F2] x [G;G;G2;G2]
NOUT = 512             # matmul free-dim tile for the output outer product

_cache = {}


RPA_C0 = -0.23549778   # minimax pair for the 1-NR bitwise-not reciprocal
RPA_C1 = 2.00173238    # (max rel err ~1.7e-3 over x*bitcast(~x) in [-4.5,-4])


def _get_rpa():
    """Register a custom DVE op: out ~= 1/(in0*in1), accum_out = sum(out).

    Product + BITWISE_NOT exponent-flip seed + one inline Newton step: six
    ALU slices, one DVE pass at 1x replacing the separate multiply,
    reciprocal and row-sum passes.  ~1.7e-3 max rel err with the minimax
    (C0, C1) pair; the row-normalized coef tolerates it (gate is 2e-2).
    """
    if "rpa" in _cache:
        return _cache["rpa"]
    import re

    import numpy as np

    from concourse import dve_ops as DO
    from concourse.dve_spec import AluOp, Bin, C0, C1, Spec, Src0, Src1, Zero
    from operator import add

    name = "RECIP_PROD_ACC_ANT"
    _x = Src0 * Src1
    _nx = Bin(AluOp.BITWISE_NOT, _x, _x)
    _y0 = _nx * C0
    _y1 = _y0 * (C1 - _x * _y0)

    def _ref(in0, in1, c0, c1, c2):
        x = (in0.astype(np.float32) * in1.astype(np.float32)).astype(np.float32)
        nx = (~x.view(np.int32)).view(np.float32)
        y0 = (nx * np.float32(c0)).astype(np.float32)
        b = (y0 * (np.float32(c1) - x * y0)).astype(np.float32)
        return b, b.reshape(b.shape[0], -1).sum(axis=-1, keepdims=True)

    spec = Spec(body=_y1, accum=add, accum_init=Zero, reference=_ref)
    shas = {}
    for ver in ("v3", "v4"):
        probe = DO.DveOp(name + "_PROBE", spec, subdim=False, uops_sha={})
        if name + "_PROBE" not in DO._SUB_OPCODE_FOR_NAME:
            DO._SUB_OPCODE_FOR_NAME[name + "_PROBE"] = 0x1F
        try:
            probe.compile(ver)
        except ValueError as e:
            m = re.search(r'"(?:v3|v4)"\]="([0-9a-f]+)"', str(e))
            if not m:
                raise
            shas[ver] = m.group(1)
    op = DO.DveOp(name, spec, subdim=False, uops_sha=shas)
    if name not in DO._SUB_OPCODE_FOR_NAME:
        DO.OPS.append(op)
        DO._SUB_OPCODE_FOR_NAME[name] = DO._CUSTOM_DVE_ROW_BASE + len(DO.OPS) - 1
        assert DO._SUB_OPCODE_FOR_NAME[name] < 0x20
    DO.CUSTOM_DVE_SPECS[name] = spec
    _cache["rpa"] = op
    return op


def _build(bench_nrep=None, bench_span="main"):
    import concourse.bacc as bacc
    import concourse.mybir as mybir
    from concourse.tile import TileContext

    rpa = _get_rpa()
    f32 = mybir.dt.float32
    f32r = mybir.dt.float32r
    Alu = mybir.AluOpType
    Act = mybir.ActivationFunctionType

    nc = bacc.Bacc(None, target_bir_lowering=False)
    g0_r = nc.declare_dram_parameter("g0_r", [K, C], f32, isOutput=False)
    g1_r = nc.declare_dram_parameter("g1_r", [K, C], f32, isOutput=False)
    f0_r = nc.declare_dram_parameter("f0_r", [K, CW], f32, isOutput=False)
    f1_r = nc.declare_dram_parameter("f1_r", [K, CW], f32, isOutput=False)
    ps2i_r = nc.declare_dram_parameter("ps2i_r", [128, NBLK], f32, isOutput=False)
    ident_r = nc.declare_dram_parameter("ident_r", [128, 128], f32, isOutput=False)
    xv = nc.declare_dram_parameter("xv", [1, CH], f32, isOutput=False)
    y = nc.declare_dram_parameter("y", [CH, CW], f32, isOutput=True)

    with TileContext(nc) as tc:
        with (
            tc.tile_pool(name="persist", bufs=1) as pp,
            tc.tile_pool(name="work", bufs=1) as wp,
            tc.tile_pool(name="psum", bufs=1, space="PSUM") as psp,
            tc.tile_pool(name="dram", bufs=1, space="DRAM") as dp,
        ):
            scr = dp.tile([128 * NBLK], f32, name="scr")
            ps2i_sb = pp.tile([128, NBLK], f32)
            nc.sync.dma_start(out=ps2i_sb[:, :], in_=ps2i_r[:, :])
            ident = pp.tile([128, 128], f32)
            nc.sync.dma_start(out=ident[:, :], in_=ident_r[:, :])
            xv_sb = pp.tile([1, CH], f32)
            nc.sync.dma_start(out=xv_sb[0:1, :], in_=xv[0:1, :])

            g0 = pp.tile([K, C], f32r)
            g1 = pp.tile([K, C], f32r)
            f0 = pp.tile([K, CW], f32r)
            f1 = pp.tile([K, CW], f32r)

            Racc = pp.tile([128, NBLK, NPCH], f32)
            Rdiag = pp.tile([128, NBLK], f32)

            def load_loop():
                # F first halves, then G chunks in consumption order,
                # then F second halves.
                nc.sync.dma_start(out=f0[:, 0:CW // 2], in_=f0_r[:, 0:CW // 2].bitcast(f32r))
                nc.sync.dma_start(out=f1[:, 0:CW // 2], in_=f1_r[:, 0:CW // 2].bitcast(f32r))
                for j in range(NPCH):
                    sl = slice(j * PCH, (j + 1) * PCH)
                    nc.sync.dma_start(out=g0[:, sl], in_=g0_r[:, sl].bitcast(f32r))
                    nc.sync.dma_start(out=g1[:, sl], in_=g1_r[:, sl].bitcast(f32r))
                nc.sync.dma_start(out=f0[:, CW // 2:], in_=f0_r[:, CW // 2:].bitcast(f32r))
                nc.sync.dma_start(out=f1[:, CW // 2:], in_=f1_r[:, CW // 2:].bitcast(f32r))

            DVE_COPY_J = ()   # which chunks' PSUM->SBUF stage runs on DVE

            def main_loop(n_lo, n_hi, parts="mcta"):
              # parts: m=matmuls, c=copies, t=tensor_tensor, a=act+diag
              for n in range(n_lo, n_hi):
                if "w" in parts:
                    fa = f0[:, 0:128]
                    fb = f1[:, 0:128]
                else:
                    fa = f0[:, n * 128:(n + 1) * 128]
                    fb = f1[:, n * 128:(n + 1) * 128]
                for j in range(NPCH):
                    sl = slice(j * PCH, (j + 1) * PCH)
                    pa = psp.tile([128, PCH], f32, tag="pa", bufs=3, name="pa")
                    pb = psp.tile([128, PCH], f32, tag="pb", bufs=3, name="pb")
                    if "m" in parts:
                        nc.tensor.matmul(
                            pa[:, :], fa, g0[:, sl],
                            start=True, stop=True,
                        )
                        nc.tensor.matmul(
                            pb[:, :], fb, g1[:, sl],
                            start=True, stop=True,
                        )
                    # stage pa to SBUF (DVE can read at most one PSUM operand)
                    sa = wp.tile([128, PCH], f32, tag="sa", bufs=3, name="sa")
                    if "c" in parts:
                        if j in DVE_COPY_J:
                            nc.vector.tensor_copy(sa[:, :], pa[:, :])
                        else:
                            nc.scalar.copy(sa[:, :], pa[:, :])
                    if "t" in parts:
                        # fused 1/(sa*pb) + row-sum accumulate, one DVE pass
                        rr = wp.tile([128, PCH], f32, tag="rr", bufs=3, name="rr")
                        nc.vector._custom_dve(
                            rpa, out=rr[:, :], in0=pb[:, :], in1=sa[:, :],
                            s0=RPA_C0, s1=RPA_C1,
                            accum_out=Racc[:, n, j:j + 1],
                        )
                        if "a" in parts and j == n // 4:
                            # extract computed 1/DENOM[i,i] (diag block sits at
                            # a static offset: host pre-rotates point columns)
                            off = 128 * (n % 4)
                            junk = wp.tile([128, 128], f32, tag="junk",
                                           bufs=2, name="junk")
                            nc.vector.affine_mul_reduce(
                                out=junk[:, :], accum_out=Rdiag[:, n:n + 1],
                                in0=rr[:, off:off + 128], in1=ident[:, :],
                                scale=1.0, bias=0.0,
                            )

            HB = NBLK // 2          # blocks per epilogue half
            HC = HB * 128           # c-columns per half

            def epilogue(half):
                nsl = slice(half * HB, (half + 1) * HB)
                Tc = pp.tile([128, HB], f32, name="Tc", tag="Tc", bufs=2)
                nc.vector.tensor_tensor(
                    Tc[:, :], Racc[:, nsl, 0], Racc[:, nsl, 1], Alu.add
                )
                for jj in range(2, NPCH):
                    nc.vector.tensor_tensor(
                        Tc[:, :], Tc[:, :], Racc[:, nsl, jj], Alu.add
                    )
                nc.vector.tensor_tensor(
                    Tc[:, :], Tc[:, :], Rdiag[:, nsl], Alu.subtract
                )
                nc.vector.tensor_tensor(
                    Tc[:, :], Tc[:, :], ps2i_sb[:, nsl], Alu.add
                )
                coef = pp.tile([128, HB], f32, name="coef", tag="coef", bufs=2)
                nc.vector.reciprocal(coef[:, :], Tc[:, :])
                nc.vector.tensor_tensor(
                    coef[:, :], coef[:, :], ps2i_sb[:, nsl], Alu.mult
                )

                # transpose coef (128, HB) -> row (1, HC) via a DRAM bounce
                nc.sync.dma_start(
                    out=scr.rearrange("(p n) -> p n", p=128)[:, nsl], in_=coef[:, :]
                )
                crow = pp.tile([1, HC], f32, name="crow", tag="crow", bufs=2)
                nc.sync.dma_start(
                    out=crow[0:1, :].rearrange("a (n p) -> a n p", n=HB),
                    in_=scr.rearrange("(p n) -> n p", n=NBLK)[nsl, :],
                )

                # y[ch, c] = x[ch] * coef[c] as K=1 outer-product matmuls
                for h in range(CH // 128):
                    for qk in range(HC // NOUT):
                        ps = psp.tile([128, NOUT], f32, tag="ps", bufs=2, name="ps")
                        nc.tensor.matmul(
                            ps[:, :],
                            xv_sb[0:1, h * 128:(h + 1) * 128],
                            crow[0:1, qk * NOUT:(qk + 1) * NOUT],
                            start=True, stop=True,
                        )
                        ysb = wp.tile([128, NOUT], f32, tag="ysb", bufs=2, name="ysb")
                        nc.scalar.copy(ysb[:, :], ps[:, :])
                        nc.sync.dma_start(
                            out=y[
                                h * 128:(h + 1) * 128,
                                half * HC + qk * NOUT: half * HC + (qk + 1) * NOUT,
                            ],
                            in_=ysb[:, :],
                        )

            def whole():
                load_loop()
                main_loop(0, NBLK // 2)
                epilogue(0)
                main_loop(NBLK // 2, NBLK)
                epilogue(1)

            if bench_nrep is None:
                whole()
            elif bench_span.startswith("main"):
                parts = bench_span[4:] or "mcta"
                load_loop()
                nc.vector.memset(Racc[:, :, :], 1.0)
                nc.vector.memset(Rdiag[:, :], 0.0)
                with tc.For_i(0, bench_nrep, 1):
                    main_loop(0, NBLK, parts=parts)
                epilogue(0)
                epilogue(1)
            elif bench_span == "load":
                with tc.For_i(0, bench_nrep, 1):
                    load_loop()
                main_loop(0, NBLK)
                epilogue(0)
                epilogue(1)
            elif bench_span == "epi":
                load_loop()
                main_loop(0, NBLK)
                with tc.For_i(0, bench_nrep, 1):
                    epilogue(0)
                    epilogue(1)
            else:
                import concourse.mybir as _mb

                with tc.For_i(
                    0, bench_nrep, 1,
                    staggered_reset=True,
                    hint_engines=(_mb.EngineType.DVE, _mb.EngineType.Activation),
                ):
                    whole()
    nc.finalize()
    return nc


def _get_nc():
    if "nc" not in _cache:
        _cache["nc"] = _build()
    return _cache["nc"]


def _trunc22(a):
    a = np.ascontiguousarray(a, dtype=np.float32)
    return (a.view(np.uint32) & np.uint32(0xFFFFF000)).view(np.float32)


def _in_maps(x, mu, sig):
    maps = []
    ident = np.eye(128, dtype=np.float32)
    for k in range(NCORES):
        b = k // 2
        half = k % 2
        sl = slice(half * CW, (half + 1) * CW)
        muc = (np.asarray(mu[b], np.float32) - np.float32(0.5)).astype(np.float32)
        mur = muc[sl]                                    # (CW, 4) row centers
        sgr = np.asarray(sig[b, sl], dtype=np.float32)   # (CW, 4)

        gs, fs = [], []
        for (da, db) in ((0, 1), (2, 3)):
            xa, xb = muc[:, da], muc[:, db]
            one = np.ones_like(xa)
            ga = [one, xa, (xa * xa).astype(np.float32)]
            gb = [one, xb, (xb * xb).astype(np.float32)]
            G = np.stack(
                [(ga[ja] * gb[jb]).astype(np.float32)
                 for ja in range(3) for jb in range(3)], 0)       # (9, C)

            def cvec(dd):
                s2 = (sgr[:, dd] * sgr[:, dd]).astype(np.float32)
                m = mur[:, dd]
                return [(s2 + m * m).astype(np.float32),
                        (np.float32(-2.0) * m).astype(np.float32),
                        np.ones_like(m)]

            ca, cb = cvec(da), cvec(db)
            F = np.stack(
                [(ca[ja] * cb[jb]).astype(np.float32)
                 for ja in range(3) for jb in range(3)], 0)       # (9, CW)
            G1 = _trunc22(G)
            F1 = _trunc22(F)
            G2 = (G - G1).astype(np.float32)
            F2 = (F - F1).astype(np.float32)
            # K segments: [F1,G1] [F2,G1] [F1,G2] [F2,G2]; hi rows are
            # pre-truncated so the DMA's fp32->fp32r rounding is an identity
            gstack = np.concatenate([G1, G1, G2, G2], 0)          # (36, C)
            fstack = np.concatenate([F1, F2, F1, F2], 0)          # (36, CW)
            # rotate points so each core's diagonal block is at col 128*n
            gstack = np.roll(gstack, -half * CW, axis=1)
            gs.append(np.ascontiguousarray(gstack))
            fs.append(np.ascontiguousarray(fstack))

        ps2 = (sgr * sgr).astype(np.float32).prod(axis=1).astype(np.float32)
        ps2i = (np.float32(1.0) / ps2).astype(np.float32)         # exact 1/DENOM_ii
        ps2i_rr = np.ascontiguousarray(ps2i.reshape(NBLK, 128).T)

        maps.append(
            {
                "g0_r": gs[0], "g1_r": gs[1],
                "f0_r": fs[0], "f1_r": fs[1],
                "ps2i_r": ps2i_rr,
                "ident_r": ident,
                "xv": np.ascontiguousarray(
                    np.asarray(x[b, :, 0], dtype=np.float32)[None, :]
                ),
            }
        )
    return maps


def kernel(x, pi, mu, sig):
    from concourse.bass_utils import run_bass_kernel_spmd

    nc = _get_nc()
    res = run_bass_kernel_spmd(nc, _in_maps(x, mu, sig), list(range(NCORES))).results
    y = np.empty((B, CH, C), np.float32)
    for k in range(NCORES):
        b = k // 2
        half = k % 2
        y[b, :, half * CW:(half + 1) * CW] = res[k]["y"]
    return y
